# revision 3
# baseline (speedup 1.0000x reference)
"""Trainium2 Bass kernel for nn_CrossAttention (B=4, NQ=NK=1024, D=1024, H=16).

Sharding: 8 cores = 4 batches x 2 head-groups (8 heads each). Per core:
  - inputs arrive pre-transposed/sliced on host (free): xqT/xkT/xvT [D, T] fp16,
    Wq/Wk/Wv column slices [D, 512] fp16, Wo row slice [512, D] fp16.
  - QT = (Wq_g)^T-free projection producing Q^T [512, T] (lhsT = W slice, rhs = xT)
  - scores computed transposed (scoresT [Tk, Tq]) so probs^T directly feeds PV
  - softmax denominators via an augmented ones-column in V (row 64 of PV output)
  - out-projection contracts head-pairs (K=128 tiles); host sums the two
    head-group partials per batch and adds the bias.
All matmuls fp16 (1 cycle/row on PE), accumulation fp32 in PSUM.
"""
import sys

sys.path.insert(0, "/opt/trn_rl_repo")

from contextlib import ExitStack

import numpy as np

import concourse.bass as bass
import concourse.tile as tile
from concourse import bacc, mybir
from concourse.bass_utils import run_bass_kernel_spmd

F32 = mybir.dt.float32
F16 = mybir.dt.float16

B, NQ, NK, D, H, HD = 4, 1024, 1024, 1024, 16, 64
NCORES = 8
HPC = 8          # heads per core
F = HPC * HD     # 512: per-core projection width
KT = D // 128    # 8 k-tiles over D
PAIRS = HPC // 2  # 4 head pairs
TKT = NK // 128  # 8 tiles over key tokens
NCH = NQ // 512  # 2 moving chunks over query tokens


def _emit(tc):
    nc = tc.nc
    ctx = ExitStack()

    xqT = nc.dram_tensor("xqT", [D, NQ], F16, kind="ExternalInput").ap()
    xkT = nc.dram_tensor("xkT", [D, NK], F16, kind="ExternalInput").ap()
    xvT = nc.dram_tensor("xvT", [D, NK], F16, kind="ExternalInput").ap()
    wq = nc.dram_tensor("wq", [D, F], F16, kind="ExternalInput").ap()
    wk = nc.dram_tensor("wk", [D, F], F16, kind="ExternalInput").ap()
    wv = nc.dram_tensor("wv", [D, F], F16, kind="ExternalInput").ap()
    wo = nc.dram_tensor("wo", [F, D], F16, kind="ExternalInput").ap()
    out = nc.dram_tensor("out", [NQ, D], F32, kind="ExternalOutput").ap()

    wpool = ctx.enter_context(tc.tile_pool(name="wpool", bufs=1))
    qkv = ctx.enter_context(tc.tile_pool(name="qkv", bufs=1))
    xin = ctx.enter_context(tc.tile_pool(name="xin", bufs=4))
    psum = ctx.enter_context(tc.tile_pool(name="psum", bufs=8, space="PSUM"))
    expool = ctx.enter_context(tc.tile_pool(name="expool", bufs=20))
    nrm = ctx.enter_context(tc.tile_pool(name="nrm", bufs=3))
    ost = ctx.enter_context(tc.tile_pool(name="ost", bufs=4))

    # ---- persistent weights ----
    wq_sb = wpool.tile([128, KT, F], F16, tag="wq")
    wk_sb = wpool.tile([128, KT, F], F16, tag="wk")
    wv_sb = wpool.tile([128, KT, F], F16, tag="wv")
    wo_sb = wpool.tile([128, PAIRS, D], F16, tag="wo")
    for k in range(KT):
        nc.sync.dma_start(out=wq_sb[:, k, :], in_=wq[k * 128:(k + 1) * 128, :])
        nc.sync.dma_start(out=wk_sb[:, k, :], in_=wk[k * 128:(k + 1) * 128, :])
        nc.sync.dma_start(out=wv_sb[:, k, :], in_=wv[k * 128:(k + 1) * 128, :])
    for p in range(PAIRS):
        nc.sync.dma_start(out=wo_sb[:, p, :], in_=wo[p * 128:(p + 1) * 128, :])

    # ---- persistent intermediates ----
    qt_sb = qkv.tile([128, PAIRS, NQ], F16, tag="qt")   # Q^T, partitions = pair dims
    kt_sb = qkv.tile([128, PAIRS, NK], F16, tag="kt")   # K^T
    vp_sb = qkv.tile([128, TKT, HPC, HD + 1], F16, tag="vp")  # V + ones col
    attT = qkv.tile([128, PAIRS, NQ], F16, tag="attT")
    nc.vector.memset(vp_sb[:, :, :, HD:HD + 1], 1.0)

    # ---- Q^T / K^T projections: out[f_tile m, q chunk n] = sum_k W[k,m]^T x xT[k,n]
    for xT, w_sb, dst, tg in ((xqT, wq_sb, qt_sb, "xq"), (xkT, wk_sb, kt_sb, "xk")):
        for n in range(NCH):
            ps = [psum.tile([128, 512], F32, tag="ps", name=f"ps_{tg}_{n}_{m}") for m in range(PAIRS)]
            for k in range(KT):
                xt = xin.tile([128, 512], F16, tag=tg)
                nc.sync.dma_start(out=xt[:], in_=xT[k * 128:(k + 1) * 128,
                                                   n * 512:(n + 1) * 512])
                for m in range(PAIRS):
                    nc.tensor.matmul(out=ps[m][:],
                                     lhsT=w_sb[:, k, m * 128:(m + 1) * 128],
                                     rhs=xt[:], start=(k == 0), stop=(k == KT - 1))
            for m in range(PAIRS):
                nc.vector.tensor_copy(out=dst[:, m, n * 512:(n + 1) * 512],
                                      in_=ps[m][:])

    # ---- V projection: out[tk_tile, f] = sum_k xvT[k, tk]^T x Wv[k, f]
    for half in range(2):
        psv = [psum.tile([128, 512], F32, tag="ps", name=f"psv_{half}_{t}") for t in range(4)]
        for k in range(KT):
            xt = xin.tile([128, 512], F16, tag="xv")
            nc.sync.dma_start(out=xt[:], in_=xvT[k * 128:(k + 1) * 128,
                                               half * 512:(half + 1) * 512])
            for t in range(4):
                nc.tensor.matmul(out=psv[t][:],
                                 lhsT=xt[:, t * 128:(t + 1) * 128],
                                 rhs=wv_sb[:, k, :], start=(k == 0), stop=(k == KT - 1))
        for t in range(4):
            tk = half * 4 + t
            nc.vector.tensor_copy(
                out=vp_sb[:, tk, :, 0:HD],
                in_=psv[t][:].rearrange("p (h d) -> p h d", h=HPC))

    # ---- attention, one head pair at a time ----
    scale = 1.0 / float(np.sqrt(HD))
    for p in range(PAIRS):
        # scoresT (transposed scores) + exp, packed 2 heads x 2 M-halves per quad
        ex = {}
        for hh in range(2):
            for tkm in range(TKT):
                ex[(hh, tkm)] = expool.tile([128, NQ], F16, tag="ex", name=f"ex_{p}_{hh}_{tkm}")
        for tkm in range(TKT):
            for n in range(NCH):
                ps_s = [psum.tile([128, 512], F32, tag="ps", name=f"ps_s_{p}_{tkm}_{n}_{i}") for i in range(2)]
                for hh in range(2):
                    r0 = hh * 64
                    for mh in range(2):
                        c0 = mh * 64
                        nc.tensor.matmul(
                            out=ps_s[hh][c0:c0 + 64, :],
                            lhsT=kt_sb[r0:r0 + 64, p,
                                       tkm * 128 + c0:tkm * 128 + c0 + 64],
                            rhs=qt_sb[r0:r0 + 64, p, n * 512:(n + 1) * 512],
                            start=True, stop=True,
                            tile_position=(r0, c0))
                for hh in range(2):
                    nc.scalar.activation(
                        out=ex[(hh, tkm)][:, n * 512:(n + 1) * 512],
                        in_=ps_s[hh][:],
                        func=mybir.ActivationFunctionType.Exp, scale=scale)

        # PV: outT'[hd+1, q] accumulated over tk tiles; row 64 = softmax denom
        for hh in range(2):
            h = p * 2 + hh
            pv_ps = []
            for n in range(NCH):
                pspv = psum.tile([65, 512], F32, tag="ps")
                for k in range(TKT):
                    nc.tensor.matmul(out=pspv[:],
                                     lhsT=vp_sb[:, k, h, :],
                                     rhs=ex[(hh, k)][:, n * 512:(n + 1) * 512],
                                     start=(k == 0), stop=(k == TKT - 1))
                pv_ps.append(pspv)

            rec = nrm.tile([1, NQ], F32, tag="rec")
            for n in range(NCH):
                nc.vector.reciprocal(out=rec[:, n * 512:(n + 1) * 512],
                                     in_=pv_ps[n][64:65, :])
            rb = nrm.tile([64, NQ], F32, tag="rb")
            nc.gpsimd.partition_broadcast(out_ap=rb[:], in_ap=rec[0:1, :], channels=64)
            if hh == 0:
                for n in range(NCH):
                    nc.vector.tensor_mul(out=attT[0:64, p, n * 512:(n + 1) * 512],
                                         in0=pv_ps[n][0:64, :],
                                         in1=rb[:, n * 512:(n + 1) * 512])
            else:
                tmp = nrm.tile([64, NQ], F16, tag="tmp")
                for n in range(NCH):
                    nc.vector.tensor_mul(out=tmp[:, n * 512:(n + 1) * 512],
                                         in0=pv_ps[n][0:64, :],
                                         in1=rb[:, n * 512:(n + 1) * 512])
                nc.sync.dma_start(out=attT[64:128, p, :], in_=tmp[:])

    # ---- output projection: out[q_tile, n] = sum_pairs attT[:, p, q]^T x Wo[p, n]
    for q in range(NQ // 128):
        for n in range(NCH):
            pso = psum.tile([128, 512], F32, tag="ps")
            for p4 in range(PAIRS):
                nc.tensor.matmul(out=pso[:],
                                 lhsT=attT[:, p4, q * 128:(q + 1) * 128],
                                 rhs=wo_sb[:, p4, n * 512:(n + 1) * 512],
                                 start=(p4 == 0), stop=(p4 == PAIRS - 1))
            ot = ost.tile([128, 512], F32, tag="ot")
            nc.vector.tensor_copy(out=ot[:], in_=pso[:])
            nc.sync.dma_start(out=out[q * 128:(q + 1) * 128,
                                      n * 512:(n + 1) * 512], in_=ot[:])
    ctx.close()


_NC_CACHE = None


def build():
    global _NC_CACHE
    if _NC_CACHE is None:
        nc = bacc.Bacc("TRN2", target_bir_lowering=False, debug=False,
                       num_devices=NCORES)
        with tile.TileContext(nc) as tc:
            _emit(tc)
        nc.compile()
        _NC_CACHE = nc
    return _NC_CACHE


def make_in_maps(inputs):
    q = np.asarray(inputs["query_tokens"], dtype=np.float32)
    kk = np.asarray(inputs["key_tokens"], dtype=np.float32)
    v = np.asarray(inputs["value_tokens"], dtype=np.float32)
    Wq = np.asarray(inputs["Wq"], dtype=np.float32)
    Wk = np.asarray(inputs["Wk"], dtype=np.float32)
    Wv = np.asarray(inputs["Wv"], dtype=np.float32)
    Wo = np.asarray(inputs["Wo"], dtype=np.float32)

    qT = [np.ascontiguousarray(q[b].T).astype(np.float16) for b in range(B)]
    kT = [np.ascontiguousarray(kk[b].T).astype(np.float16) for b in range(B)]
    vT = [np.ascontiguousarray(v[b].T).astype(np.float16) for b in range(B)]
    wq_g = [np.ascontiguousarray(Wq[:, g * F:(g + 1) * F]).astype(np.float16)
            for g in range(2)]
    wk_g = [np.ascontiguousarray(Wk[:, g * F:(g + 1) * F]).astype(np.float16)
            for g in range(2)]
    wv_g = [np.ascontiguousarray(Wv[:, g * F:(g + 1) * F]).astype(np.float16)
            for g in range(2)]
    wo_g = [np.ascontiguousarray(Wo[g * F:(g + 1) * F, :]).astype(np.float16)
            for g in range(2)]

    in_maps = []
    for c in range(NCORES):
        b, g = c // 2, c % 2
        in_maps.append({
            "xqT": qT[b], "xkT": kT[b], "xvT": vT[b],
            "wq": wq_g[g], "wk": wk_g[g], "wv": wv_g[g], "wo": wo_g[g],
        })
    return in_maps


def combine(results, bo):
    out = np.zeros((B, NQ, D), dtype=np.float32)
    for c in range(NCORES):
        out[c // 2] += results[c]["out"]
    out += np.asarray(bo, dtype=np.float32)[None, None, :]
    return out


def kernel(**inputs):
    nc = build()
    in_maps = make_in_maps(inputs)
    res = run_bass_kernel_spmd(nc, in_maps, list(range(NCORES)))
    return combine(res.results, inputs["bo"])
